# revision 1
# baseline (speedup 1.0000x reference)
"""Self-contained Trainium2 kernel for ReRoPE sparse attention.

Problem: x(2,1024,2048) -> attention with 16 Q heads / 8 KV heads (GQA),
RoPE within a 256-token causal band, ReRoPE (query rotated at fixed
position 256, keys unrotated) outside the band, -> out proj (2048x2048).

Sharding: 8 cores = 2 batches x 4 head groups. Each core computes 4 Q
heads / 2 KV heads of one batch plus its slice of all projections, and
produces a partial (1024,2048) output (wo row-parallel). Partials are
summed on the host (the per-batch all-reduce equivalent).

Score identity used: s2 = (R_W q)@k  ==  q @ (R_{-W} k), so the fixed
ReRoPE rotation is applied once to K instead of Q (q2 is just raw q).
Head dims are de-interleaved (evens|odds) via a host-side permutation of
wq/wk columns so RoPE pairs live on partitions (p, p+64).

All device compute in bf16 (fp32 PSUM accumulation).
"""

import numpy as np
import ml_dtypes

B, S, D = 2, 1024, 2048
NH, NKV, HD = 16, 8, 128
W = 256
HPC, KPC = 4, 2            # q heads / kv heads per core
KC = D // 128              # 16 contraction chunks
SB = S // 128              # 8 sequence blocks
SCALE = 1.0 / float(np.sqrt(HD))
BF16 = ml_dtypes.bfloat16

_NC_CACHE = {}


def _build_nc():
    import concourse.bass as bass
    import concourse.tile as tile
    from concourse import bacc, mybir
    from contextlib import ExitStack

    bf = mybir.dt.bfloat16
    f32 = mybir.dt.float32
    AF = mybir.ActivationFunctionType

    nc = bacc.Bacc()
    xt = nc.declare_dram_parameter("xt", [D, S], bf, isOutput=False)
    wq = nc.declare_dram_parameter("wq", [D, HPC * HD], bf, isOutput=False)
    wk = nc.declare_dram_parameter("wk", [D, KPC * HD], bf, isOutput=False)
    wv = nc.declare_dram_parameter("wv", [D, KPC * HD], bf, isOutput=False)
    wo = nc.declare_dram_parameter("wo", [HPC * HD, D], bf, isOutput=False)
    tab = nc.declare_dram_parameter("tab", [128, 2 * S], bf, isOutput=False)
    cst = nc.declare_dram_parameter("cst", [128, 3 * 128], bf, isOutput=False)
    cwd = nc.declare_dram_parameter("cw", [128, 2], f32, isOutput=False)
    out = nc.declare_dram_parameter("out", [S, D], bf, isOutput=True)

    with tile.TileContext(nc) as tc:
        with ExitStack() as ctx:
            p_x = ctx.enter_context(tc.tile_pool(name="p_x", bufs=1))
            p_w = ctx.enter_context(tc.tile_pool(name="p_w", bufs=1))
            p_tab = ctx.enter_context(tc.tile_pool(name="p_tab", bufs=1))
            p_q = ctx.enter_context(tc.tile_pool(name="p_q", bufs=2 * HPC))
            p_k = ctx.enter_context(tc.tile_pool(name="p_k", bufs=2 * KPC))
            p_v = ctx.enter_context(tc.tile_pool(name="p_v", bufs=SB))
            p_ao = ctx.enter_context(tc.tile_pool(name="p_ao", bufs=HPC))
            p_e = ctx.enter_context(tc.tile_pool(name="p_e", bufs=12))
            p_pt = ctx.enter_context(tc.tile_pool(name="p_pt", bufs=12))
            p_rt = ctx.enter_context(tc.tile_pool(name="p_rt", bufs=14))
            p_kr = ctx.enter_context(tc.tile_pool(name="p_kr", bufs=2))
            p_rc = ctx.enter_context(tc.tile_pool(name="p_rc", bufs=4))
            p_st = ctx.enter_context(tc.tile_pool(name="p_st", bufs=4))

            ps_proj = ctx.enter_context(
                tc.tile_pool(name="ps_proj", bufs=2, space="PSUM"))
            ps_attn = ctx.enter_context(
                tc.tile_pool(name="ps_attn", bufs=4, space="PSUM"))
            ps_out = ctx.enter_context(
                tc.tile_pool(name="ps_out", bufs=2, space="PSUM"))

            # ---- input DMAs: few big transfers, split over 2 HWDGE rings
            xt_sb = p_x.tile([128, KC * S], bf, tag="xt")
            xt_d = xt.ap().rearrange("(t p) s -> p t s", p=128)
            xt_v = xt_sb[:].rearrange("p (t s) -> p t s", t=KC)
            wk_sb = p_w.tile([128, KC * KPC * HD], bf, tag="wk")
            wk_d = wk.ap().rearrange("(t p) c -> p t c", p=128)
            wk_v = wk_sb[:].rearrange("p (t c) -> p t c", t=KC)
            wq_sb = p_w.tile([128, KC * HPC * HD], bf, tag="wq")
            wq_d = wq.ap().rearrange("(t p) c -> p t c", p=128)
            wq_v = wq_sb[:].rearrange("p (t c) -> p t c", t=KC)
            wv_sb = p_w.tile([128, KC * KPC * HD], bf, tag="wv")
            wv_d = wv.ap().rearrange("(t p) c -> p t c", p=128)
            wv_v = wv_sb[:].rearrange("p (t c) -> p t c", t=KC)
            wo_sb = p_w.tile([128, HPC * D], bf, tag="wo")
            wo_d = wo.ap().rearrange("(t p) c -> p t c", p=128)
            wo_v = wo_sb[:].rearrange("p (t c) -> p t c", t=HPC)

            # two HWDGE rings in parallel; x split across both
            nc.sync.dma_start(wk_v[:, 0:2, :], wk_d[:, 0:2, :])
            nc.scalar.dma_start(xt_v[:, 8:10, :], xt_d[:, 8:10, :])
            nc.sync.dma_start(xt_v[:, 0:1, :], xt_d[:, 0:1, :])
            nc.scalar.dma_start(xt_v[:, 10:12, :], xt_d[:, 10:12, :])
            nc.sync.dma_start(xt_v[:, 1:2, :], xt_d[:, 1:2, :])
            nc.sync.dma_start(wk_v[:, 2:8, :], wk_d[:, 2:8, :])
            nc.sync.dma_start(xt_v[:, 2:4, :], xt_d[:, 2:4, :])
            nc.scalar.dma_start(xt_v[:, 12:16, :], xt_d[:, 12:16, :])
            nc.scalar.dma_start(wk_v[:, 8:16, :], wk_d[:, 8:16, :])
            nc.sync.dma_start(xt_v[:, 4:6, :], xt_d[:, 4:6, :])
            nc.sync.dma_start(xt_v[:, 6:8, :], xt_d[:, 6:8, :])
            for c in range(2):
                nc.sync.dma_start(wq_v[:, 8 * c:8 * c + 8, :],
                                  wq_d[:, 8 * c:8 * c + 8, :])
            # scalar ring: everything else
            tab_sb = p_tab.tile([128, 2 * S], bf, tag="tab")
            nc.scalar.dma_start(tab_sb[:], tab[:, :])
            cst_sb = p_tab.tile([128, 3 * 128], bf, tag="cst")
            nc.scalar.dma_start(cst_sb[:], cst[:, :])
            cw_sb = p_tab.tile([128, 2], f32, tag="cw")
            nc.scalar.dma_start(cw_sb[:], cwd[:, :])
            nc.scalar.dma_start(wv_v[:, 0:8, :], wv_d[:, 0:8, :])
            nc.scalar.dma_start(wv_v[:, 8:16, :], wv_d[:, 8:16, :])
            nc.scalar.dma_start(wo_v[:, 0:2, :], wo_d[:, 0:2, :])
            nc.scalar.dma_start(wo_v[:, 2:4, :], wo_d[:, 2:4, :])

            def xts(t, lo, hi):
                return xt_sb[:, t * S + lo:t * S + hi]

            cosT = tab_sb[:, 0:S]
            sinT = tab_sb[:, S:2 * S]     # top half negated (host-side)
            cwv = cw_sb[:, 0:1]
            swv = cw_sb[:, 1:2]           # bottom half negated (host-side)
            m0_t = cst_sb[:, 0:128]      # (k <= q)
            m2_t = cst_sb[:, 128:256]    # (q < k)
            id_t = cst_sb[:, 256:384]

            def rope_var(dst, src, half):
                """Positional rope; pairs on (p, p+64). Table halves carry
                the signs: sinT[0:64] = -sin, sinT[64:128] = +sin."""
                sl = slice(half * 512, (half + 1) * 512)
                qr, qi = src[0:64, sl], src[64:128, sl]
                t1 = p_rt.tile([64, 512], bf, tag="rt")
                nc.vector.tensor_mul(t1[:], qr, cosT[0:64, sl])
                t2 = p_rt.tile([64, 512], bf, tag="rt")
                nc.vector.tensor_mul(t2[:], qi, sinT[64:128, sl])
                nc.vector.tensor_sub(dst[0:64, sl], t1[:], t2[:])
                t3 = p_rt.tile([64, 512], bf, tag="rt")
                nc.vector.tensor_mul(t3[:], qr, sinT[0:64, sl])
                t4 = p_rt.tile([64, 512], bf, tag="rt")
                nc.vector.tensor_mul(t4[:], qi, cosT[64:128, sl])
                nc.vector.tensor_sub(dst[64:128, sl], t4[:], t3[:])

            def rope_negw(dst, src, half):
                """R_{-W}: or = r*cw + i*sw, oi = i*cw - r*sw.
                cw col1: top = +sw, bottom = -sw."""
                sl = slice(half * 512, (half + 1) * 512)
                kr_, ki_ = src[0:64, sl], src[64:128, sl]
                t1 = p_rt.tile([64, 512], bf, tag="rt")
                nc.vector.tensor_scalar_mul(t1[:], kr_, cw_sb[0:64, 0:1])
                t2 = p_rt.tile([64, 512], bf, tag="rt")
                nc.vector.tensor_scalar_mul(t2[:], ki_, cw_sb[64:128, 1:2])
                nc.vector.tensor_sub(dst[0:64, sl], t1[:], t2[:])
                t3 = p_rt.tile([64, 512], bf, tag="rt")
                nc.vector.tensor_scalar_mul(t3[:], ki_, cw_sb[64:128, 0:1])
                t4 = p_rt.tile([64, 512], bf, tag="rt")
                nc.vector.tensor_scalar_mul(t4[:], kr_, cw_sb[0:64, 1:2])
                nc.vector.tensor_sub(dst[64:128, sl], t3[:], t4[:])

            # ---- K projection -> k1 (roped), k2p (R_{-W} k) ----
            # t-major in pairs of psum groups so PE streams with the DMA
            k1_t, k2_t = [], []
            for kv in range(KPC):
                d1 = p_k.tile([128, S], bf, tag="k")
                d2 = p_k.tile([128, S], bf, tag="k")
                pss = [ps_proj.tile([128, 512], f32, tag="proj",
                                    name=f"kps{kv}{half}")
                       for half in range(2)]
                for t in range(KC):
                    for half in range(2):
                        nc.tensor.matmul(
                            pss[half][:],
                            lhsT=wk_sb[:, t * 256 + kv * 128:
                                       t * 256 + (kv + 1) * 128],
                            rhs=xts(t, half * 512, (half + 1) * 512),
                            start=(t == 0), stop=(t == KC - 1))
                kr = p_kr.tile([128, 1024], bf, tag="kr")
                for half in range(2):
                    nc.scalar.copy(kr[:, half * 512:(half + 1) * 512],
                                   pss[half][:])
                    rope_var(d1, kr, half)
                    rope_negw(d2, kr, half)
                k1_t.append(d1)
                k2_t.append(d2)

            # ---- Q projection -> q1 (roped), q2 (raw cast) ----
            q1_t, q2_t = [], []
            for h in range(HPC):
                d1 = p_q.tile([128, S], bf, tag="q")
                d2 = p_q.tile([128, S], bf, tag="q")
                pss = [ps_proj.tile([128, 512], f32, tag="proj",
                                    name=f"qps{h}{half}")
                       for half in range(2)]
                for t in range(KC):
                    for half in range(2):
                        nc.tensor.matmul(
                            pss[half][:],
                            lhsT=wq_sb[:, t * 512 + h * 128:
                                       t * 512 + (h + 1) * 128],
                            rhs=xts(t, half * 512, (half + 1) * 512),
                            start=(t == 0), stop=(t == KC - 1))
                for half in range(2):
                    nc.scalar.copy(d2[:, half * 512:(half + 1) * 512],
                                   pss[half][:])
                    rope_var(d1, d2, half)
                q1_t.append(d1)
                q2_t.append(d2)

            # ---- V projection (natural [s, hd] + ones columns) ----
            v_t = []
            for sb in range(SB):
                ps = ps_proj.tile([128, KPC * HD], f32, tag="proj")
                for t in range(KC):
                    nc.tensor.matmul(
                        ps[:],
                        lhsT=xts(t, sb * 128, (sb + 1) * 128),
                        rhs=wv_sb[:, t * 256:(t + 1) * 256],
                        start=(t == 0), stop=(t == KC - 1))
                tv = p_v.tile([128, 2 * (HD + 1)], bf, tag="v")
                nc.vector.tensor_copy(tv[:, 0:HD], ps[:, 0:HD])
                nc.vector.tensor_copy(tv[:, HD + 1:2 * HD + 1],
                                      ps[:, HD:2 * HD])
                nc.vector.memset(tv[:, HD:HD + 1], 1.0)
                nc.vector.memset(tv[:, 2 * HD + 1:2 * HD + 2], 1.0)
                v_t.append(tv)

            # ---- attention + out-proj, row (query-block) major ----
            ao_t = [p_ao.tile([128, S], bf, tag="ao", name=f"ao{h}")
                    for h in range(HPC)]

            def flush(pend):
                if not pend:
                    return
                row = pend[0][1]
                for (h, i, an) in pend:
                    pst = ps_out.tile([128, 128], bf, tag="out")
                    nc.tensor.transpose(pst[:], an[:], id_t)
                    nc.vector.tensor_copy(
                        ao_t[h][:, i * 128:(i + 1) * 128], pst[:])
                for cg in range(4):
                    ps = ps_out.tile([128, 512], f32, tag="out")
                    for hc in range(HPC):
                        nc.tensor.matmul(
                            ps[:],
                            lhsT=ao_t[hc][:, row * 128:(row + 1) * 128],
                            rhs=wo_sb[:, hc * D + cg * 512:
                                      hc * D + (cg + 1) * 512],
                            start=(hc == 0), stop=(hc == HPC - 1))
                    st = p_st.tile([128, 512], bf, tag="st")
                    if cg % 2 == 0:
                        nc.vector.tensor_copy(st[:], ps[:])
                    else:
                        nc.scalar.copy(st[:], ps[:])
                    nc.sync.dma_start(
                        out[row * 128:(row + 1) * 128,
                            cg * 512:(cg + 1) * 512], st[:])

            def emit_scores(i, h):
                kv = h // 2
                j0 = max(0, i - 2)
                nb = i - j0 + 1
                qs1 = q1_t[h][:, i * 128:(i + 1) * 128]
                qs2 = q2_t[h][:, i * 128:(i + 1) * 128]
                # band scores s1^T blocks j0..i  -> one wide psum
                psb = ps_attn.tile([128, nb * 128], f32, tag="attn")
                for m in range(nb):
                    j = j0 + m
                    nc.tensor.matmul(
                        psb[:, m * 128:(m + 1) * 128],
                        lhsT=k1_t[kv][:, j * 128:(j + 1) * 128],
                        rhs=qs1, start=True, stop=True)
                eb = p_e.tile([128, nb * 128], bf, tag="e")
                nc.scalar.activation(eb[:], psb[:], AF.Exp, scale=SCALE)
                p0 = p_pt.tile([128, 128], bf, tag="pt")
                nc.vector.tensor_mul(
                    p0[:], eb[:, (nb - 1) * 128:nb * 128], m0_t)
                # far scores s2^T blocks 0..i-2 -> wide psums
                efs = []          # (j_start, width_blocks, exp tile)
                j = 0
                while j <= i - 2:
                    wseg = min(4, i - 1 - j)
                    psf = ps_attn.tile([128, wseg * 128], f32, tag="attn")
                    for m in range(wseg):
                        nc.tensor.matmul(
                            psf[:, m * 128:(m + 1) * 128],
                            lhsT=k2_t[kv][:, (j + m) * 128:
                                          (j + m + 1) * 128],
                            rhs=qs2, start=True, stop=True)
                    ef = p_e.tile([128, wseg * 128], bf, tag="e")
                    nc.scalar.activation(ef[:], psf[:], AF.Exp,
                                         scale=SCALE)
                    efs.append((j, wseg, ef))
                    j += wseg

                # dual-block select (post-exp masking)
                pd = None
                if i >= 2:
                    js, ws, ef = efs[-1]
                    fsl = ef[:, (ws - 1) * 128:ws * 128]
                    pa = p_pt.tile([128, 128], bf, tag="pt")
                    nc.vector.tensor_mul(pa[:], eb[:, 0:128], m2_t)
                    pd = p_pt.tile([128, 128], bf, tag="pt")
                    nc.vector.tensor_mul(pd[:], fsl, m0_t)
                    nc.vector.tensor_add(pd[:], pd[:], pa[:])

                def P(j):
                    if j == i:
                        return p0[:]
                    if j == i - 2 and i >= 2:
                        return pd[:]
                    if j >= j0:
                        m = j - j0
                        return eb[:, m * 128:(m + 1) * 128]
                    for (js, ws, ef) in efs:
                        if js <= j < js + ws:
                            m = j - js
                            return ef[:, m * 128:(m + 1) * 128]
                    raise AssertionError
                return P

            def emit_attnv(i, h, P):
                kv = h // 2
                pso = ps_proj.tile([128, HD + 1], f32, tag="proj")
                for j in range(i + 1):
                    nc.tensor.matmul(
                        pso[:], lhsT=P(j),
                        rhs=v_t[j][:, kv * (HD + 1):(kv + 1) * (HD + 1)],
                        start=(j == 0), stop=(j == i))
                rc = p_rc.tile([128, 1], f32, tag="rc")
                nc.vector.reciprocal(rc[:], pso[:, HD:HD + 1])
                an = p_pt.tile([128, 128], bf, tag="an")
                nc.vector.tensor_scalar_mul(an[:], pso[:, 0:HD], rc[:])
                pend.append((h, i, an))

            # scores lead attnv by one head; transposes + out-proj lag a row
            prev, pend = [], []
            for i in range(SB):
                pending = None
                for h in range(HPC):
                    Pf = emit_scores(i, h)
                    if pending is not None:
                        emit_attnv(*pending)
                    pending = (i, h, Pf)
                if prev:
                    flush(prev)
                    prev = []
                emit_attnv(*pending)
                prev, pend = pend, []
            flush(prev)

    nc.finalize()
    return nc


def _get_nc():
    if "nc" not in _NC_CACHE:
        _NC_CACHE["nc"] = _build_nc()
    return _NC_CACHE["nc"]


def _host_inputs(x, freqs_cos, freqs_sin, wq, wk, wv, wo):
    """Build the 8 per-core input maps (host-side shard + layout prep)."""
    x = np.asarray(x, np.float32)
    wq = np.asarray(wq, np.float32)
    wk = np.asarray(wk, np.float32)
    wv = np.asarray(wv, np.float32)
    wo = np.asarray(wo, np.float32)
    perm = np.concatenate([np.arange(0, HD, 2), np.arange(1, HD, 2)])

    cos_t = np.asarray(freqs_cos, np.float32).T        # (64, S)
    sin_t = np.asarray(freqs_sin, np.float32).T
    top = np.concatenate([cos_t, -sin_t], axis=1)      # (64, 2S)
    bot = np.concatenate([cos_t, sin_t], axis=1)
    tab = np.ascontiguousarray(np.concatenate([top, bot], axis=0)).astype(BF16)
    ki = np.arange(128)[:, None]
    qi = np.arange(128)[None, :]
    m0 = (ki <= qi).astype(BF16)                       # causal / far-select
    m2 = (qi < ki).astype(BF16)                        # in-band select (d=2)
    ident = np.eye(128, dtype=BF16)
    cstm = np.ascontiguousarray(np.concatenate([m0, m2, ident], axis=1))

    wq3 = wq.reshape(D, NH, HD)
    wk3 = wk.reshape(D, NKV, HD)
    wv3 = wv.reshape(D, NKV, HD)
    wo3 = wo.reshape(NH, HD, D)

    in_maps = []
    for c in range(8):
        b, g = divmod(c, 4)
        wqc = wq3[:, 4 * g:4 * g + 4][:, :, perm].reshape(D, HPC * HD)
        wkc = wk3[:, 2 * g:2 * g + 2][:, :, perm].reshape(D, KPC * HD)
        wvc = wv3[:, 2 * g:2 * g + 2].reshape(D, KPC * HD)
        woc = wo3[4 * g:4 * g + 4].reshape(HPC * HD, D)
        in_maps.append({
            "xt": np.ascontiguousarray(x[b].T).astype(BF16),
            "wq": np.ascontiguousarray(wqc).astype(BF16),
            "wk": np.ascontiguousarray(wkc).astype(BF16),
            "wv": np.ascontiguousarray(wvc).astype(BF16),
            "wo": np.ascontiguousarray(woc).astype(BF16),
            "tab": tab, "cst": cstm,
            "cw": np.ascontiguousarray(np.concatenate([
                np.stack([cos_t[:, W], sin_t[:, W]], axis=1),
                np.stack([cos_t[:, W], -sin_t[:, W]], axis=1),
            ], axis=0)).astype(np.float32),
        })
    return in_maps


def _run(nc, in_maps, **kw):
    from concourse.bass_utils import run_bass_kernel_spmd
    return run_bass_kernel_spmd(nc, in_maps, core_ids=list(range(8)), **kw)


def kernel(x, freqs_cos, freqs_sin, wq, wk, wv, wo):
    nc = _get_nc()
    in_maps = _host_inputs(x, freqs_cos, freqs_sin, wq, wk, wv, wo)
    res = _run(nc, in_maps)
    parts = [np.asarray(res.results[c]["out"], np.float32) for c in range(8)]
    out = np.stack([sum(parts[0:4]), sum(parts[4:8])])
    return out.astype(np.float32)

